# revision 1
# baseline (speedup 1.0000x reference)
"""Tensor-parallel decoder layer on 8 TRN2 NeuronCores.

Sharding:
  - Attention: 16 heads -> 2 per core. Per-core partial attn_out is
    ReduceScattered (fp16) so core c owns rows [256c, 256c+256).
  - Global LayerNorm (scalar mean/var over the whole [S,E] tensor):
    per-core partial (sum, sumsq) AllReduced as a tiny fp32 tensor.
  - FFN: hidden dim 8192 -> 1024 per core; partial [S,E] output
    ReduceScattered per 512-column chunk (fp16), overlapping FFN2.
  - h is AllGathered transposed (bf16) since every matmul contracting
    over E needs h^T as the moving operand.

Matmul layout notes (PE computes out = lhsT.T @ rhs, contraction on the
partition dim):
  - x^T resident in SBUF (bf16) feeds Q/K/V projections.
  - scores are built transposed: S^T[t,s] tiles, so exp(S^T) tiles feed
    attn@v directly as lhsT with no transposes; softmax normalization is
    deferred: rowsum via a ones-column matmul, applied as a per-partition
    scale on the PSUM->SBUF copy of attn@v output.
"""

import math
import sys

sys.path.insert(0, "/opt/trn_rl_repo")

import numpy as np
import ml_dtypes

_bf16 = ml_dtypes.bfloat16

import concourse.bass as bass
import concourse.mybir as mybir
import concourse.tile as tile
from concourse import bacc
from concourse.bass_utils import run_bass_kernel_spmd

S, E, H, KD, FF = 2048, 2048, 16, 128, 8192
EPS = 1e-5
NCORES = 8
HPC = H // NCORES          # heads per core = 2
FSH = FF // NCORES         # ffn hidden shard = 1024
RROWS = S // NCORES        # row shard = 256
NTOT = float(S * E)
ISCALE = 1.0 / math.sqrt(KD)

F32 = mybir.dt.float32
BF16 = mybir.dt.bfloat16
F16 = mybir.dt.float16
AF = mybir.ActivationFunctionType
AL = mybir.AluOpType
AX = mybir.AxisListType

# packed triangular offsets for eT tiles: tile(tc, sb) at TRI[sb] + tc
TRI = [0, 4, 12, 24]
NTRI = 40


def _build():
    nc = bacc.Bacc(
        "TRN2",
        target_bir_lowering=False,
        debug=False,
        enable_asserts=True,
        num_devices=NCORES,
    )

    # ---- external I/O (per-core shards prepared on the host) ----
    xtb_d = nc.dram_tensor("xtb", [128, 16, S], BF16, kind="ExternalInput")
    wq_d = nc.dram_tensor("wqt", [HPC, 128, 16, KD], F32, kind="ExternalInput")
    wk_d = nc.dram_tensor("wkt", [HPC, 128, 16, KD], F32, kind="ExternalInput")
    wv_d = nc.dram_tensor("wvt", [HPC, 128, 16, E], F32, kind="ExternalInput")
    w1_d = nc.dram_tensor("w1t", [128, 8, 2048], F32, kind="ExternalInput")
    w2_d = nc.dram_tensor("w2t", [128, 8, 4, 512], F32, kind="ExternalInput")
    bq_d = nc.dram_tensor("bqs", [128, HPC], F32, kind="ExternalInput")
    bk_d = nc.dram_tensor("bks", [128, HPC], F32, kind="ExternalInput")
    b1_d = nc.dram_tensor("b1s", [128, 8], F32, kind="ExternalInput")
    yb1_d = nc.dram_tensor("yb1", [128, E], F32, kind="ExternalInput")
    yb2_d = nc.dram_tensor("yb2", [128, E], F32, kind="ExternalInput")
    xr_d = nc.dram_tensor("xr", [RROWS, E], F32, kind="ExternalInput")
    lng_d = nc.dram_tensor("lngr", [RROWS, E], BF16, kind="ExternalInput")
    lnb_d = nc.dram_tensor("lnbr", [RROWS, E], BF16, kind="ExternalInput")
    mask_d = nc.dram_tensor("mask", [4, 128, 512], F32, kind="ExternalInput")
    id_d = nc.dram_tensor("ident", [128, 128], F32, kind="ExternalInput")
    ones_d = nc.dram_tensor("ones", [128, 8], F32, kind="ExternalInput")
    onesr_d = nc.dram_tensor("onesr", [1, 128], F32, kind="ExternalInput")
    out_d = nc.dram_tensor("out", [RROWS, E], F32, kind="ExternalOutput")

    RG = [list(range(NCORES))]

    with tile.TileContext(nc) as tc:
        with (
            tc.tile_pool(name="persist", bufs=1) as pp,
            tc.tile_pool(name="dram", bufs=1, space="DRAM") as dp,
            tc.tile_pool(name="ps512", bufs=4, space="PSUM") as ps512,
            tc.tile_pool(name="psT", bufs=2, space="PSUM") as psT,
            tc.tile_pool(name="psR", bufs=2, space="PSUM") as psR,
        ):
            # ---- collective bounce buffers (internal DRAM) ----
            att_in = [
                [
                    dp.tile([S, FSH], F16, name=f"att_in_{h}_{fh}", tag=f"ati{h}{fh}")
                    for fh in range(2)
                ]
                for h in range(HPC)
            ]
            att_out = [
                [
                    dp.tile(
                        [RROWS, FSH],
                        F16,
                        name=f"att_out_{h}_{fh}",
                        tag=f"ato{h}{fh}",
                    )
                    for fh in range(2)
                ]
                for h in range(HPC)
            ]
            st1_in = dp.tile([1, 8], F32, name="st1_in", tag="st1i")
            st1_out = dp.tile([1, 8], F32, name="st1_out", tag="st1o", addr_space="Shared")
            st2_in = dp.tile([1, 8], F32, name="st2_in", tag="st2i")
            st2_out = dp.tile([1, 8], F32, name="st2_out", tag="st2o", addr_space="Shared")
            ag_in = [
                dp.tile([RROWS, E // 2], BF16, name=f"ag_in{j}", tag=f"agi{j}")
                for j in range(2)
            ]
            ag_out = [
                dp.tile([S, E // 2], BF16, name=f"ag_out{j}", tag=f"ago{j}", addr_space="Shared")
                for j in range(2)
            ]
            ffn_in = [
                dp.tile([S, 512], F16, name=f"ffn_in_{eb}", tag=f"ffi{eb}")
                for eb in range(4)
            ]
            ffn_out = [
                dp.tile([RROWS, 512], F16, name=f"ffn_out_{eb}", tag=f"ffo{eb}")
                for eb in range(4)
            ]

            # ---- persistent small tiles ----
            ident = pp.tile([128, 128], F32, name="ident")
            nc.sync.dma_start(ident[:], id_d[:])
            onesc = pp.tile([128, 8], F32, name="onesc")
            nc.sync.dma_start(onesc[:], ones_d[:])
            onesr = pp.tile([1, 128], F32, name="onesr")
            nc.sync.dma_start(onesr[:], onesr_d[:])
            ones_bf = pp.tile([128, 1], BF16, name="ones_bf")
            nc.vector.tensor_copy(out=ones_bf[:], in_=onesc[:, 0:1])
            bq_sb = pp.tile([128, HPC], F32, name="bq_sb")
            nc.sync.dma_start(bq_sb[:], bq_d[:])
            bk_sb = pp.tile([128, HPC], F32, name="bk_sb")
            nc.sync.dma_start(bk_sb[:], bk_d[:])
            b1_sb = pp.tile([128, 8], F32, name="b1_sb")
            nc.sync.dma_start(b1_sb[:], b1_d[:])
            qkT = pp.tile([128, 2, HPC, S], BF16, name="qkT")  # [d, q/k, head, s]
            recips = pp.tile([128, HPC, 16], F32, name="recips")

            # =========== phase 0 + attention ===========
            with tc.tile_pool(name="attn", bufs=1) as ap_:
                xT = ap_.tile([128, 16, S], BF16, name="xT")  # x^T, e on partitions
                maskb = ap_.tile([128, 4, 512], BF16, name="maskb")
                with tc.tile_pool(name="prep", bufs=3) as prep, nc.named_scope("prep"):
                    # HAM warmup: ~8us of back-to-back matmuls to unthrottle PE
                    wtile = prep.tile([128, 512], BF16, name="wtile", tag="wtile", bufs=1)
                    nc.vector.memset(wtile[:], 0.0)
                    for _w in range(24):
                        pw = ps512.tile([128, 512], F32, name="pw", tag="p512")
                        nc.tensor.matmul(pw[:], wtile[:, :128], wtile[:], start=True, stop=True)
                    maskf = prep.tile([128, 4, 512], F32, name="maskf", tag="maskf", bufs=1)
                    nc.sync.dma_start(maskf[:], mask_d.ap().rearrange("j p s -> p j s"))
                    nc.vector.tensor_copy(out=maskb[:], in_=maskf[:])
                    # x^T arrives pre-transposed/pre-cast from the host: 16 DMAs
                    # (one per eo) so Q/K accumulation can start as chunks land
                    for eo in range(16):
                        nc.sync.dma_start(xT[:, eo, :], xtb_d[:, eo, :])

                # ---- Q/K projections for both heads (scaled/biased) ----
                with tc.tile_pool(name="qkw", bufs=2) as qkw, nc.named_scope("qkproj"):
                    for h in range(HPC):
                        for qi, (w_d, b_sb, scl) in enumerate(
                            ((wq_d, bq_sb, ISCALE), (wk_d, bk_sb, 1.0))
                        ):
                            wf = qkw.tile([128, 16, KD], F32, name="wf", tag="wf")
                            nc.sync.dma_start(wf[:], w_d[h])
                            wb = qkw.tile([128, 16, KD], BF16, name="wb", tag="wb")
                            nc.vector.tensor_copy(out=wb[:], in_=wf[:])
                            pqs = [
                                ps512.tile([128, 512], F32, name=f"pq{sb}", tag="p512")
                                for sb in range(4)
                            ]
                            for eo in range(16):
                                for sb in range(4):
                                    nc.tensor.matmul(
                                        pqs[sb][:],
                                        wb[:, eo, :],
                                        xT[:, eo, sb * 512 : (sb + 1) * 512],
                                        start=(eo == 0),
                                        stop=(eo == 15),
                                    )
                            for sb in range(4):
                                nc.scalar.activation(
                                    qkT[:, qi, h, sb * 512 : (sb + 1) * 512],
                                    pqs[sb][:],
                                    AF.Identity,
                                    bias=b_sb[:, h : h + 1],
                                    scale=scl,
                                )

                # ---- per-head attention ----
                eT = ap_.tile([128, NTRI, 512], BF16, name="eT")
                v_sb = ap_.tile([128, 16, FSH], BF16, name="v_sb")
                with (
                    tc.tile_pool(name="wvp", bufs=3) as wvp,
                    tc.tile_pool(name="wvb", bufs=1) as wvbp,
                    tc.tile_pool(name="astg", bufs=4) as astg,
                ):
                    for h in range(HPC):
                      with nc.named_scope(f"scores{h}"):
                        for sb in range(4):
                            for tcn in range(4 * sb + 4):
                                psc = ps512.tile([128, 512], F32, name="psc", tag="p512")
                                nc.tensor.matmul(
                                    psc[:],
                                    qkT[:, 1, h, tcn * 128 : (tcn + 1) * 128],
                                    qkT[:, 0, h, sb * 512 : (sb + 1) * 512],
                                    start=True,
                                    stop=True,
                                )
                                dst = eT[:, TRI[sb] + tcn, :]
                                if tcn >= 4 * sb:
                                    etmp = astg.tile(
                                        [128, 512], BF16, name="etmp", tag="etmp"
                                    )
                                    nc.scalar.activation(etmp[:], psc[:], AF.Exp)
                                    nc.vector.tensor_tensor(
                                        dst, etmp[:], maskb[:, tcn - 4 * sb, :], AL.mult
                                    )
                                else:
                                    nc.scalar.activation(dst, psc[:], AF.Exp)

                        # pass B: per f-half: v-projection then attn@v
                        for fh in range(2):
                          with nc.named_scope(f"vproj{h}{fh}"):
                            for fb in range(2):
                                wvb = wvbp.tile([128, 16, 512], BF16, name="wvb", tag="wvb")
                                for eo in range(16):
                                    wvf = wvp.tile([128, 512], F32, name="wvf", tag="wvf")
                                    nc.sync.dma_start(
                                        wvf[:],
                                        wv_d[
                                            h,
                                            :,
                                            eo,
                                            fh * 1024 + fb * 512 : fh * 1024 + (fb + 1) * 512,
                                        ],
                                    )
                                    nc.vector.tensor_copy(out=wvb[:, eo, :], in_=wvf[:])
                                for tcn in range(16):
                                    pv = ps512.tile([128, 512], F32, name="pv", tag="p512")
                                    for eo in range(16):
                                        nc.tensor.matmul(
                                            pv[:],
                                            xT[:, eo, tcn * 128 : (tcn + 1) * 128],
                                            wvb[:, eo, :],
                                            start=(eo == 0),
                                            stop=(eo == 15),
                                        )
                                    nc.vector.tensor_copy(
                                        out=v_sb[:, tcn, fb * 512 : (fb + 1) * 512],
                                        in_=pv[:],
                                    )

                          with nc.named_scope(f"attnv{h}{fh}"):
                            for i in range(15, -1, -1):
                                sb, so = i // 4, (i % 4) * 128
                                pa = [
                                    ps512.tile([128, 512], F32, name=f"pa{fb}", tag="p512")
                                    for fb in range(2)
                                ]
                                if fh == 0:
                                    pr = psR.tile([128, 1], F32, name="pr", tag="pr")
                                for tcn in range(i + 1):
                                    lhs = eT[:, TRI[sb] + tcn, so : so + 128]
                                    for fb in range(2):
                                        nc.tensor.matmul(
                                            pa[fb][:],
                                            lhs,
                                            v_sb[:, tcn, fb * 512 : (fb + 1) * 512],
                                            start=(tcn == 0),
                                            stop=(tcn == i),
                                        )
                                    if fh == 0:
                                        nc.tensor.matmul(
                                            pr[:],
                                            lhs,
                                            ones_bf[:],
                                            start=(tcn == 0),
                                            stop=(tcn == i),
                                        )
                                if fh == 0:
                                    rsf = astg.tile([128, 1], F32, name="rsf", tag="rsf")
                                    nc.vector.tensor_copy(out=rsf[:], in_=pr[:])
                                    nc.vector.reciprocal(recips[:, h, i : i + 1], rsf[:])
                                stg = astg.tile([128, 1024], F16, name="stg", tag="stg")
                                for fb in range(2):
                                    nc.scalar.activation(
                                        stg[:, fb * 512 : (fb + 1) * 512],
                                        pa[fb][:],
                                        AF.Copy,
                                        scale=recips[:, h, i : i + 1],
                                    )
                                nc.sync.dma_start(
                                    att_in[h][fh][i * 128 : (i + 1) * 128, :], stg[:]
                                )
                            nc.gpsimd.collective_compute(
                                "ReduceScatter",
                                AL.add,
                                replica_groups=RG,
                                ins=[att_in[h][fh][:]],
                                outs=[att_out[h][fh][:]],
                            )

            # =========== LN1 (global mean/var) ===========
            with tc.tile_pool(name="mid", bufs=1) as midp:
              h_own = midp.tile([128, 2, E], BF16, name="h_own")
              with tc.tile_pool(name="ln1", bufs=1) as lp, nc.named_scope("ln1"):
                  ys = lp.tile([128, 2, E], F32, name="ys")
                  yb1t = lp.tile([128, E], F32, name="yb1t")
                  nc.sync.dma_start(yb1t[:], yb1_d[:])
                  for rt in range(2):
                      xrt = lp.tile([128, E], F32, name="xrt", tag="xrt", bufs=2)
                      nc.sync.dma_start(xrt[:], xr_d[rt * 128 : (rt + 1) * 128, :])
                      nc.vector.tensor_tensor(ys[:, rt, :], xrt[:], yb1t[:], AL.add)
                      for h in range(HPC):
                          for fh in range(2):
                              rof = lp.tile([128, FSH], F16, name="rof", tag="rof", bufs=2)
                              nc.sync.dma_start(
                                  rof[:], att_out[h][fh][rt * 128 : (rt + 1) * 128, :]
                              )
                              dstv = ys[:, rt, fh * FSH : (fh + 1) * FSH]
                              nc.vector.tensor_tensor(dstv, dstv, rof[:], AL.add)

                  _stats_ln(nc, tc, lp, psT, ys, onesc, onesr, st1_in, st1_out, RG)
                  bc = _ln_scalars(nc, lp, psT, onesr, st1_out)
                  lngt = midp.tile([128, 2, E], BF16, name="lngt")
                  nc.sync.dma_start(lngt[:], lng_d.ap().rearrange("(t p) e -> p t e", p=128))
                  lnbt = midp.tile([128, 2, E], BF16, name="lnbt")
                  nc.sync.dma_start(lnbt[:], lnb_d.ap().rearrange("(t p) e -> p t e", p=128))
                  ht_f32 = lp.tile([128, E], F32, name="ht_f32", tag="htf", bufs=2)
                  for rt in range(2):
                      nc.scalar.activation(
                          ht_f32[:],
                          ys[:, rt, :],
                          AF.Identity,
                          bias=bc[:, 0:1],
                          scale=bc[:, 1:2],
                      )
                      nc.vector.tensor_tensor(
                          ht_f32[:], ht_f32[:], lngt[:, rt, :], AL.mult
                      )
                      nc.vector.tensor_tensor(
                          h_own[:, rt, :], ht_f32[:], lnbt[:, rt, :], AL.add
                      )
                  agb = h_own
                  for j in range(2):
                      nc.sync.dma_start(
                          ag_in[j].rearrange("(t p) e -> p t e", p=128),
                          agb[:, :, j * (E // 2) : (j + 1) * (E // 2)],
                      )
                      nc.gpsimd.collective_compute(
                          "AllGather",
                          AL.bypass,
                          replica_groups=RG,
                          ins=[ag_in[j][:]],
                          outs=[ag_out[j][:]],
                      )

              # =========== FFN (hidden shard 1024) ===========
              with tc.tile_pool(name="ffn", bufs=1) as fp, nc.named_scope("ffn"):
                  hT = fp.tile([128, 16, S], BF16, name="hT")
                  zT = fp.tile([128, 8, S], BF16, name="zT")
                  with tc.tile_pool(name="wst", bufs=2) as wst:
                      hidb = wst.tile([128, 128], BF16, name="hidb", tag="hidb", bufs=1)
                      nc.vector.tensor_copy(out=hidb[:], in_=ident[:])
                      for j in range(2):
                          for st in range(16):
                              hrow = wst.tile([128, E // 2], BF16, name="hrow", tag="hrow", bufs=3)
                              nc.sync.dma_start(
                                  hrow[:], ag_out[j][st * 128 : (st + 1) * 128, :]
                              )
                              for eh in range(8):
                                  eo = j * 8 + eh
                                  pth = psT.tile([128, 128], BF16, name="pth", tag="pt")
                                  nc.tensor.transpose(
                                      pth[:], hrow[:, eh * 128 : (eh + 1) * 128], hidb[:]
                                  )
                                  if eo % 2 == 0:
                                      nc.vector.tensor_copy(
                                          out=hT[:, eo, st * 128 : (st + 1) * 128], in_=pth[:]
                                      )
                                  else:
                                      nc.scalar.copy(
                                          hT[:, eo, st * 128 : (st + 1) * 128], pth[:]
                                      )
                      for ft in range(8):
                          w1f = wst.tile([128, 2048], F32, name="w1f", tag="w1f", bufs=1)
                          nc.sync.dma_start(w1f[:], w1_d[:, ft, :])
                          w1b = wst.tile([128, 16, KD], BF16, name="w1b", tag="w1b")
                          nc.vector.tensor_copy(
                              out=w1b[:], in_=w1f.rearrange("p (eo f) -> p eo f", eo=16)
                          )
                          pzs = [
                              ps512.tile([128, 512], F32, name=f"pz{sb}", tag="p512")
                              for sb in range(4)
                          ]
                          for eo in range(16):
                              for sb in range(4):
                                  nc.tensor.matmul(
                                      pzs[sb][:],
                                      w1b[:, eo, :],
                                      hT[:, eo, sb * 512 : (sb + 1) * 512],
                                      start=(eo == 0),
                                      stop=(eo == 15),
                                  )
                          for sb in range(4):
                              nc.scalar.activation(
                                  zT[:, ft, sb * 512 : (sb + 1) * 512],
                                  pzs[sb][:],
                                  AF.Relu,
                                  bias=b1_sb[:, ft : ft + 1],
                              )
                      for eb in range(4):
                          w2b = wst.tile([128, 8, 512], BF16, name="w2b", tag="w2b")
                          for wh in range(2):
                              w2f = wst.tile([128, 4, 512], F32, name="w2f", tag="w2f", bufs=1)
                              nc.sync.dma_start(w2f[:], w2_d[:, wh * 4 : (wh + 1) * 4, eb, :])
                              nc.vector.tensor_copy(out=w2b[:, wh * 4 : (wh + 1) * 4, :], in_=w2f[:])
                          for i in range(15, -1, -1):
                              pf = ps512.tile([128, 512], F32, name="pf", tag="p512")
                              for fc in range(8):
                                  nc.tensor.matmul(
                                      pf[:],
                                      zT[:, fc, i * 128 : (i + 1) * 128],
                                      w2b[:, fc, :],
                                      start=(fc == 0),
                                      stop=(fc == 7),
                                  )
                              fstg = wst.tile([128, 512], F16, name="fstg", tag="fstg", bufs=4)
                              nc.scalar.activation(fstg[:], pf[:], AF.Copy)
                              nc.sync.dma_start(
                                  ffn_in[eb][i * 128 : (i + 1) * 128, :], fstg[:]
                              )
                          nc.gpsimd.collective_compute(
                              "ReduceScatter",
                              AL.add,
                              replica_groups=RG,
                              ins=[ffn_in[eb][:]],
                              outs=[ffn_out[eb][:]],
                          )

              # =========== LN2 + output ===========
              with tc.tile_pool(name="ln2", bufs=1) as l2, nc.named_scope("ln2"):
                  ys2 = l2.tile([128, 2, E], F32, name="ys2")
                  yb2t = l2.tile([128, E], F32, name="yb2t")
                  nc.sync.dma_start(yb2t[:], yb2_d[:])
                  for rt in range(2):
                      nc.vector.tensor_tensor(
                          ys2[:, rt, :], h_own[:, rt, :], yb2t[:], AL.add
                      )
                      for eb in range(4):
                          fot = l2.tile([128, 512], F16, name="fot", tag="fot", bufs=2)
                          nc.sync.dma_start(
                              fot[:], ffn_out[eb][rt * 128 : (rt + 1) * 128, :]
                          )
                          dstv = ys2[:, rt, eb * 512 : (eb + 1) * 512]
                          nc.vector.tensor_tensor(dstv, dstv, fot[:], AL.add)

                  _stats_ln(nc, tc, l2, psT, ys2, onesc, onesr, st2_in, st2_out, RG)
                  bc2 = _ln_scalars(nc, l2, psT, onesr, st2_out)
                  lngt2 = lngt
                  lnbt2 = lnbt
                  for rt in range(2):
                      ot = l2.tile([128, E], F32, name="ot", tag="ot", bufs=2)
                      nc.scalar.activation(
                          ot[:],
                          ys2[:, rt, :],
                          AF.Identity,
                          bias=bc2[:, 0:1],
                          scale=bc2[:, 1:2],
                      )
                      nc.vector.tensor_tensor(ot[:], ot[:], lngt2[:, rt, :], AL.mult)
                      nc.vector.tensor_tensor(ot[:], ot[:], lnbt2[:, rt, :], AL.add)
                      nc.sync.dma_start(out_d[rt * 128 : (rt + 1) * 128, :], ot[:])

    nc.compile()
    return nc


def _stats_ln(nc, tc, pool, psT, ys, onesc, onesr, st_in, st_out, RG):
    """partial sum/sumsq of ys [128, 2, E] -> tiny fp32 AllReduce.

    Computed per (row-tile, column-half) so each partial only depends on the
    ReduceScatter chunks feeding that half (starts before the last RS lands).
    """
    parts = pool.tile([128, 8], F32, name="parts", tag="parts")
    sqs = pool.tile([128, E // 2], BF16, name="sqs", tag="sqs")
    for rt in range(2):
        for ch in range(2):
            idx = rt * 2 + ch
            ysl = ys[:, rt, ch * (E // 2) : (ch + 1) * (E // 2)]
            nc.vector.tensor_reduce(parts[:, idx : idx + 1], ysl, axis=AX.X, op=AL.add)
            nc.scalar.activation(
                sqs[:], ysl, AF.Square, accum_out=parts[:, 4 + idx : 5 + idx]
            )
    pstat = psT.tile([128, 128], F32, name="pstat", tag="pt")
    nc.tensor.matmul(pstat[:1, :8], onesc[:, 0:1], parts[:], start=True, stop=True)
    st4s = pool.tile([1, 8], F32, name="st4s", tag="st4s")
    nc.vector.tensor_copy(out=st4s[:], in_=pstat[:1, :8])
    st4 = pool.tile([1, 8], F32, name="st4", tag="st4")
    nc.vector.memset(st4[:], 0.0)
    nc.vector.tensor_reduce(st4[:, 0:1], st4s[:, 0:4], axis=AX.X, op=AL.add)
    nc.vector.tensor_reduce(st4[:, 1:2], st4s[:, 4:8], axis=AX.X, op=AL.add)
    nc.sync.dma_start(st_in[:], st4[:])
    nc.gpsimd.collective_compute(
        "AllReduce", AL.add, replica_groups=RG, ins=[st_in[:]], outs=[st_out[:]]
    )


def _ln_scalars(nc, pool, psT, onesr, st_out):
    """AllReduced (sum, sumsq) -> bc [128, 2] = (-m*rstd, rstd) broadcast."""
    so = pool.tile([1, 8], F32, name="so", tag="so")
    nc.sync.dma_start(so[:], st_out[:])
    sc = pool.tile([1, 8], F32, name="sc", tag="sc")
    # sc0 = m, sc1 = E[y^2], sc2 = m^2, sc3 = var, sc4 = rstd, sc5 = -m*rstd
    nc.scalar.mul(sc[:, 0:1], so[:, 0:1], 1.0 / NTOT)
    nc.scalar.mul(sc[:, 1:2], so[:, 1:2], 1.0 / NTOT)
    nc.scalar.activation(sc[:, 2:3], sc[:, 0:1], AF.Square)
    nc.vector.tensor_tensor(sc[:, 3:4], sc[:, 1:2], sc[:, 2:3], AL.subtract)
    nc.vector.tensor_scalar_add(sc[:, 2:3], sc[:, 3:4], EPS)  # var + eps
    # rstd = exp(-0.5 * ln(var + eps)) (keeps ACT on the exp/ln table)
    nc.scalar.activation(sc[:, 6:7], sc[:, 2:3], AF.Ln)
    nc.scalar.activation(sc[:, 4:5], sc[:, 6:7], AF.Exp, scale=-0.5)
    nc.vector.tensor_tensor(sc[:, 7:8], sc[:, 0:1], sc[:, 4:5], AL.mult)
    nc.scalar.mul(sc[:, 5:6], sc[:, 7:8], -1.0)
    s2 = pool.tile([1, 2], F32, name="s2", tag="s2")
    nc.vector.tensor_copy(out=s2[:, 0:1], in_=sc[:, 5:6])
    nc.vector.tensor_copy(out=s2[:, 1:2], in_=sc[:, 4:5])
    pb = psT.tile([128, 128], F32, name="pb", tag="pt")
    nc.tensor.matmul(pb[:, :2], onesr[:], s2[:], start=True, stop=True)
    bc = pool.tile([128, 2], F32, name="bc", tag="bc")
    nc.vector.tensor_copy(out=bc[:], in_=pb[:, :2])
    return bc


_NC_CACHE = None


def _get_nc():
    global _NC_CACHE
    if _NC_CACHE is None:
        _NC_CACHE = _build()
    return _NC_CACHE


def _prep_core(c, inputs):
    f32 = np.float32
    x = np.ascontiguousarray(inputs["input"], dtype=f32)
    Wq, Wk, Wv = inputs["Wq"], inputs["Wk"], inputs["Wv"]
    bq, bk, bv = inputs["bq"], inputs["bk"], inputs["bv"]
    W1, b1, W2, b2 = inputs["W1"], inputs["b1"], inputs["W2"], inputs["b2"]
    ln_g, ln_b = inputs["ln_g"], inputs["ln_b"]
    h0 = c * HPC
    wqt = np.ascontiguousarray(
        np.stack(
            [Wq[h0 + h].reshape(16, 128, KD).transpose(1, 0, 2) for h in range(HPC)]
        ),
        dtype=f32,
    )
    wkt = np.ascontiguousarray(
        np.stack(
            [Wk[h0 + h].reshape(16, 128, KD).transpose(1, 0, 2) for h in range(HPC)]
        ),
        dtype=f32,
    )
    wvt = np.ascontiguousarray(
        np.stack(
            [Wv[h0 + h].reshape(16, 128, E).transpose(1, 0, 2) for h in range(HPC)]
        ),
        dtype=f32,
    )
    W1s = W1[:, c * FSH : (c + 1) * FSH]
    w1t = np.ascontiguousarray(
        W1s.reshape(16, 128, 8, 128).transpose(1, 2, 0, 3).reshape(128, 8, 2048),
        dtype=f32,
    )
    W2s = W2[c * FSH : (c + 1) * FSH, :]
    w2t = np.ascontiguousarray(
        W2s.reshape(8, 128, 4, 512).transpose(1, 0, 2, 3), dtype=f32
    )
    bqs = np.ascontiguousarray((bq[h0 : h0 + HPC] * ISCALE).T, dtype=f32)
    bks = np.ascontiguousarray(bk[h0 : h0 + HPC].T, dtype=f32)
    b1s = np.ascontiguousarray(b1[c * FSH : (c + 1) * FSH].reshape(8, 128).T, dtype=f32)
    yb1 = np.ascontiguousarray(np.broadcast_to(bv.sum(axis=0), (128, E)), dtype=f32)
    yb2 = np.ascontiguousarray(np.broadcast_to(b2, (128, E)), dtype=f32)
    rows = slice(c * RROWS, (c + 1) * RROWS)
    jj, tp, sf = np.meshgrid(
        np.arange(4), np.arange(128), np.arange(512), indexing="ij"
    )
    mask = ((128 * jj + tp) <= sf).astype(f32)
    xtb = np.ascontiguousarray(
        x.T.reshape(16, 128, S).transpose(1, 0, 2).astype(_bf16)
    )
    return {
        "xtb": xtb,
        "wqt": wqt,
        "wkt": wkt,
        "wvt": wvt,
        "w1t": w1t,
        "w2t": w2t,
        "bqs": bqs,
        "bks": bks,
        "b1s": b1s,
        "yb1": yb1,
        "yb2": yb2,
        "xr": np.ascontiguousarray(x[rows], dtype=f32),
        "lngr": np.ascontiguousarray(np.asarray(ln_g[rows], dtype=f32).astype(_bf16)),
        "lnbr": np.ascontiguousarray(np.asarray(ln_b[rows], dtype=f32).astype(_bf16)),
        "mask": np.ascontiguousarray(mask),
        "ident": np.eye(128, dtype=f32),
        "ones": np.ones((128, 8), dtype=f32),
        "onesr": np.ones((1, 128), dtype=f32),
    }


def kernel(**inputs):
    nc = _get_nc()
    inputs = {k: np.asarray(v, dtype=np.float32) for k, v in inputs.items()}
    in_maps = [_prep_core(c, inputs) for c in range(NCORES)]
    res = run_bass_kernel_spmd(nc, in_maps, core_ids=list(range(NCORES)))
    out = np.concatenate([res.results[c]["out"] for c in range(NCORES)], axis=0)
    return np.ascontiguousarray(out, dtype=np.float32)



# revision 9
# speedup vs baseline: 1.0409x; 1.0409x over previous
"""Tensor-parallel decoder layer on 8 TRN2 NeuronCores.

Sharding:
  - Attention: 16 heads -> 2 per core. Per-core partial attn_out is
    ReduceScattered (fp16) so core c owns rows [256c, 256c+256).
  - Global LayerNorm (scalar mean/var over the whole [S,E] tensor):
    per-core partial (sum, sumsq) AllReduced as a tiny fp32 tensor.
  - y = x + attn_out is AllGathered TRANSPOSED and PRE-affine (so the
    first half's AllGather starts during attention and the stats
    AllReduce overlaps the second half's AllGather); each core then
    applies the LN1 affine on the full y^T locally (redundant work on
    otherwise-idle engines) to produce h^T for the FFN.
  - FFN: hidden dim 8192 -> 1024 per core; partial [S,E] output
    ReduceScattered per 512-column chunk (fp16), overlapping FFN2.

Matmul layout notes (PE computes out = lhsT.T @ rhs, contraction on the
partition dim):
  - x^T resident in SBUF (bf16) feeds Q/K/V projections.
  - scores are built transposed: S^T[t,s] tiles, so exp(S^T) tiles feed
    attn@v directly as lhsT with no transposes; softmax normalization is
    deferred: rowsum via a ones-column matmul, applied as a per-partition
    scale on the PSUM->SBUF copy of attn@v output.
  - All weights are pre-cast to bf16 on the host (no on-chip casts).
"""

import math
import sys

sys.path.insert(0, "/opt/trn_rl_repo")

import numpy as np
import ml_dtypes

_bf16 = ml_dtypes.bfloat16

import concourse.bass as bass
import concourse.mybir as mybir
import concourse.tile as tile
from concourse import bacc
from concourse.bass_utils import run_bass_kernel_spmd

S, E, H, KD, FF = 2048, 2048, 16, 128, 8192
EPS = 1e-5
NCORES = 8
HPC = H // NCORES          # heads per core = 2
FSH = FF // NCORES         # ffn hidden shard = 1024
RROWS = S // NCORES        # row shard = 256
NTOT = float(S * E)
ISCALE = 1.0 / math.sqrt(KD)

F32 = mybir.dt.float32
BF16 = mybir.dt.bfloat16
F16 = mybir.dt.float16
AF = mybir.ActivationFunctionType
AL = mybir.AluOpType
AX = mybir.AxisListType

# packed triangular offsets for eT tiles: tile(tc, sb) at TRI[sb] + tc
TRI = [0, 4, 12, 24]
NTRI = 40


def _build():
    nc = bacc.Bacc(
        "TRN2",
        target_bir_lowering=False,
        debug=False,
        enable_asserts=True,
        num_devices=NCORES,
    )

    # ---- external I/O (per-core shards prepared on the host) ----
    xtb_d = nc.dram_tensor("xtb", [128, 16, S], BF16, kind="ExternalInput")
    wq_d = nc.dram_tensor("wqt", [HPC, 128, 16, KD], BF16, kind="ExternalInput")
    wk_d = nc.dram_tensor("wkt", [HPC, 128, 16, KD], BF16, kind="ExternalInput")
    wv_d = nc.dram_tensor("wvt", [HPC, 128, 16, E], BF16, kind="ExternalInput")
    w1_d = nc.dram_tensor("w1t", [128, 8, 2048], BF16, kind="ExternalInput")
    w2_d = nc.dram_tensor("w2t", [128, 8, 4, 512], BF16, kind="ExternalInput")
    bq_d = nc.dram_tensor("bqs", [128, HPC], F32, kind="ExternalInput")
    bk_d = nc.dram_tensor("bks", [128, HPC], F32, kind="ExternalInput")
    b1_d = nc.dram_tensor("b1s", [128, 8], F32, kind="ExternalInput")
    yb1_d = nc.dram_tensor("yb1", [128, E], F32, kind="ExternalInput")
    yb2_d = nc.dram_tensor("yb2", [128, E], F32, kind="ExternalInput")
    xr_d = nc.dram_tensor("xr", [RROWS, E], F32, kind="ExternalInput")
    lng_d = nc.dram_tensor("lngr", [RROWS, E], BF16, kind="ExternalInput")
    lnb_d = nc.dram_tensor("lnbr", [RROWS, E], BF16, kind="ExternalInput")
    gT_d = nc.dram_tensor("gT", [128, 16, S], BF16, kind="ExternalInput")
    bT_d = nc.dram_tensor("bT", [128, 16, S], BF16, kind="ExternalInput")
    mask_d = nc.dram_tensor("mask", [128, 4, 512], BF16, kind="ExternalInput")
    id_d = nc.dram_tensor("ident", [128, 128], BF16, kind="ExternalInput")
    ones_d = nc.dram_tensor("ones", [128, 8], F32, kind="ExternalInput")
    onesr_d = nc.dram_tensor("onesr", [1, 128], F32, kind="ExternalInput")
    out_d = nc.dram_tensor("out", [RROWS, E], F32, kind="ExternalOutput")

    RG = [list(range(NCORES))]

    with tile.TileContext(nc) as tc:
        with (
            tc.tile_pool(name="persist", bufs=1) as pp,
            tc.tile_pool(name="dram", bufs=1, space="DRAM") as dp,
            tc.tile_pool(name="ps512", bufs=4, space="PSUM") as ps512,
            tc.tile_pool(name="psT", bufs=2, space="PSUM") as psT,
            tc.tile_pool(name="psR", bufs=2, space="PSUM") as psR,
        ):
            # ---- collective bounce buffers (internal DRAM) ----
            att_in = [
                [
                    dp.tile([S, FSH], F16, name=f"att_in_{h}_{fh}", tag=f"ati{h}{fh}")
                    for fh in range(2)
                ]
                for h in range(HPC)
            ]
            att_out = [
                [
                    dp.tile(
                        [RROWS, FSH],
                        F16,
                        name=f"att_out_{h}_{fh}",
                        tag=f"ato{h}{fh}",
                    )
                    for fh in range(2)
                ]
                for h in range(HPC)
            ]
            st1_in = dp.tile([1, 8], F32, name="st1_in", tag="st1i")
            st1_out = dp.tile([1, 8], F32, name="st1_out", tag="st1o", addr_space="Shared")
            st2_in = dp.tile([1, 8], F32, name="st2_in", tag="st2i")
            st2_out = dp.tile([1, 8], F32, name="st2_out", tag="st2o", addr_space="Shared")
            # transposed-y AllGather: per fh-half, in = own yT columns
            # [E/2 rows, 256 s], out = concat over cores -> [8*E/2, 256]
            agt_in = [
                dp.tile([E // 2, RROWS], BF16, name=f"agt_in{j}", tag=f"agi{j}")
                for j in range(2)
            ]
            agt_out = [
                dp.tile(
                    [NCORES * (E // 2), RROWS],
                    BF16,
                    name=f"agt_out{j}",
                    tag=f"ago{j}",
                    addr_space="Shared",
                )
                for j in range(2)
            ]
            ffn_in = [
                dp.tile([S, 512], F16, name=f"ffn_in_{eb}", tag=f"ffi{eb}")
                for eb in range(4)
            ]
            ffn_out = [
                dp.tile([RROWS, 512], F16, name=f"ffn_out_{eb}", tag=f"ffo{eb}")
                for eb in range(4)
            ]

            # ---- persistent small tiles ----
            ident = pp.tile([128, 128], BF16, name="ident")
            nc.sync.dma_start(ident[:], id_d[:])
            onesc = pp.tile([128, 8], F32, name="onesc")
            nc.sync.dma_start(onesc[:], ones_d[:])
            onesr = pp.tile([1, 128], F32, name="onesr")
            nc.sync.dma_start(onesr[:], onesr_d[:])
            ones_bf = pp.tile([128, 1], BF16, name="ones_bf")
            nc.vector.tensor_copy(out=ones_bf[:], in_=onesc[:, 0:1])
            bq_sb = pp.tile([128, HPC], F32, name="bq_sb")
            nc.sync.dma_start(bq_sb[:], bq_d[:])
            bk_sb = pp.tile([128, HPC], F32, name="bk_sb")
            nc.sync.dma_start(bk_sb[:], bk_d[:])
            b1_sb = pp.tile([128, 8], F32, name="b1_sb")
            nc.sync.dma_start(b1_sb[:], b1_d[:])
            qkT = pp.tile([128, 2, HPC, S], BF16, name="qkT")  # [d, q/k, head, s]
            recips = pp.tile([128, HPC, 16], F32, name="recips")

            # =========== phase 0 + attention ===========
            with tc.tile_pool(name="attn", bufs=1) as ap_:
                xT = ap_.tile([128, 16, S], BF16, name="xT")  # x^T, e on partitions
                maskb = ap_.tile([128, 4, 512], BF16, name="maskb")
                with tc.tile_pool(name="qkw", bufs=1) as qkw:
                  wqk = [
                      [
                          qkw.tile([128, 16, KD], BF16, name=f"wqk{h}{qi}", tag=f"wqk{h}{qi}")
                          for qi in range(2)
                      ]
                      for h in range(HPC)
                  ]
                  # weight + x DMAs first so they hit the queues early
                  for h in range(HPC):
                      nc.sync.dma_start(wqk[h][0][:], wq_d[h])
                      nc.sync.dma_start(wqk[h][1][:], wk_d[h])
                  for eo in range(16):
                      nc.sync.dma_start(xT[:, eo, :], xtb_d[:, eo, :])
                  nc.sync.dma_start(maskb[:], mask_d[:])
                  with tc.tile_pool(name="prep", bufs=3) as prep, nc.named_scope("prep"):
                    # HAM warmup: ~8us of back-to-back matmuls to unthrottle PE
                    wtile = prep.tile([128, 512], BF16, name="wtile", tag="wtile", bufs=1)
                    nc.vector.memset(wtile[:], 0.0)
                    for _w in range(24):
                        pw = ps512.tile([128, 512], F32, name="pw", tag="p512")
                        nc.tensor.matmul(pw[:], wtile[:, :128], wtile[:], start=True, stop=True)

                  # ---- Q/K projections for both heads (scaled/biased) ----
                  with nc.named_scope("qkproj"):
                    for h in range(HPC):
                        for qi, (b_sb, scl) in enumerate(
                            ((bq_sb, ISCALE), (bk_sb, 1.0))
                        ):
                            wb = wqk[h][qi]
                            pqs = [
                                ps512.tile([128, 512], F32, name=f"pq{sb}", tag="p512")
                                for sb in range(4)
                            ]
                            for eo in range(16):
                                for sb in range(4):
                                    nc.tensor.matmul(
                                        pqs[sb][:],
                                        wb[:, eo, :],
                                        xT[:, eo, sb * 512 : (sb + 1) * 512],
                                        start=(eo == 0),
                                        stop=(eo == 15),
                                    )
                            for sb in range(4):
                                nc.scalar.activation(
                                    qkT[:, qi, h, sb * 512 : (sb + 1) * 512],
                                    pqs[sb][:],
                                    AF.Identity,
                                    bias=b_sb[:, h : h + 1],
                                    scale=scl,
                                )

                # ---- per-head attention ----
                eT = ap_.tile([128, NTRI, 512], BF16, name="eT")
                v_sb = ap_.tile([128, 16, FSH], BF16, name="v_sb")
                with (
                    tc.tile_pool(name="wvb", bufs=2) as wvbp,
                    tc.tile_pool(name="astg", bufs=4) as astg,
                ):
                    for h in range(HPC):
                      with nc.named_scope(f"scores{h}"):
                        for sb in range(4):
                            for tcn in range(4 * sb + 4):
                                psc = ps512.tile([128, 512], F32, name="psc", tag="p512")
                                nc.tensor.matmul(
                                    psc[:],
                                    qkT[:, 1, h, tcn * 128 : (tcn + 1) * 128],
                                    qkT[:, 0, h, sb * 512 : (sb + 1) * 512],
                                    start=True,
                                    stop=True,
                                )
                                dst = eT[:, TRI[sb] + tcn, :]
                                if tcn >= 4 * sb:
                                    etmp = astg.tile(
                                        [128, 512], BF16, name="etmp", tag="etmp", bufs=3
                                    )
                                    nc.scalar.activation(etmp[:], psc[:], AF.Exp)
                                    nc.vector.tensor_tensor(
                                        dst, etmp[:], maskb[:, tcn - 4 * sb, :], AL.mult
                                    )
                                else:
                                    nc.scalar.activation(dst, psc[:], AF.Exp)

                        # pass B: per f-half: v-projection then attn@v
                        for fh in range(2):
                          with nc.named_scope(f"vproj{h}{fh}"):
                            for fb in range(2):
                                wvb = wvbp.tile([128, 16, 512], BF16, name="wvb", tag="wvb")
                                nc.sync.dma_start(
                                    wvb[:],
                                    wv_d[
                                        h,
                                        :,
                                        :,
                                        fh * 1024 + fb * 512 : fh * 1024 + (fb + 1) * 512,
                                    ],
                                )
                                for tcn in range(16):
                                    pv = ps512.tile([128, 512], F32, name="pv", tag="p512")
                                    for eo in range(16):
                                        nc.tensor.matmul(
                                            pv[:],
                                            xT[:, eo, tcn * 128 : (tcn + 1) * 128],
                                            wvb[:, eo, :],
                                            start=(eo == 0),
                                            stop=(eo == 15),
                                        )
                                    nc.vector.tensor_copy(
                                        out=v_sb[:, tcn, fb * 512 : (fb + 1) * 512],
                                        in_=pv[:],
                                    )

                          with nc.named_scope(f"attnv{h}{fh}"):
                            for i in range(15, -1, -1):
                                sb, so = i // 4, (i % 4) * 128
                                pa = [
                                    ps512.tile([128, 512], F32, name=f"pa{fb}", tag="p512")
                                    for fb in range(2)
                                ]
                                if fh == 0:
                                    pr = psR.tile([128, 1], F32, name="pr", tag="pr")
                                for tcn in range(i + 1):
                                    lhs = eT[:, TRI[sb] + tcn, so : so + 128]
                                    for fb in range(2):
                                        nc.tensor.matmul(
                                            pa[fb][:],
                                            lhs,
                                            v_sb[:, tcn, fb * 512 : (fb + 1) * 512],
                                            start=(tcn == 0),
                                            stop=(tcn == i),
                                        )
                                    if fh == 0:
                                        nc.tensor.matmul(
                                            pr[:],
                                            lhs,
                                            ones_bf[:],
                                            start=(tcn == 0),
                                            stop=(tcn == i),
                                        )
                                if fh == 0:
                                    rsf = astg.tile([128, 1], F32, name="rsf", tag="rsf")
                                    nc.vector.tensor_copy(out=rsf[:], in_=pr[:])
                                    nc.vector.reciprocal(recips[:, h, i : i + 1], rsf[:])
                                stg = astg.tile([128, 1024], F16, name="stg", tag="stg")
                                for fb in range(2):
                                    nc.scalar.activation(
                                        stg[:, fb * 512 : (fb + 1) * 512],
                                        pa[fb][:],
                                        AF.Copy,
                                        scale=recips[:, h, i : i + 1],
                                    )
                                nc.sync.dma_start(
                                    att_in[h][fh][i * 128 : (i + 1) * 128, :], stg[:]
                                )
                            nc.gpsimd.collective_compute(
                                "ReduceScatter",
                                AL.add,
                                replica_groups=RG,
                                ins=[att_in[h][fh][:]],
                                outs=[att_out[h][fh][:]],
                            )

            # =========== LN1 (global mean/var) + transposed-y AllGather ===========
            with tc.tile_pool(name="mid", bufs=1) as midp:
              hT = midp.tile([128, 16, S], BF16, name="hT")  # y^T then h^T
              h_own = midp.tile([128, 2, E], BF16, name="h_own")
              with tc.tile_pool(name="ln1", bufs=1) as lp, nc.named_scope("ln1"):
                  ys = lp.tile([128, 2, E], F32, name="ys")
                  yb1t = lp.tile([128, E], F32, name="yb1t")
                  nc.sync.dma_start(yb1t[:], yb1_d[:])
                  hidb = lp.tile([128, 128], BF16, name="hidb")
                  nc.vector.tensor_copy(out=hidb[:], in_=ident[:])
                  for rt in range(2):
                      xrt = lp.tile([128, E], F32, name="xrt", tag="xrt", bufs=1)
                      nc.sync.dma_start(xrt[:], xr_d[rt * 128 : (rt + 1) * 128, :])
                      nc.vector.tensor_tensor(ys[:, rt, :], xrt[:], yb1t[:], AL.add)

                  ysb = lp.tile([128, 2, 2, E // 2], BF16, name="ysb")  # [p, rt, fh, s]
                  for fh in range(2):
                      cols = slice(fh * (E // 2), (fh + 1) * (E // 2))
                      for rt in range(2):
                          for h in range(HPC):
                              rof = lp.tile([128, FSH], F16, name="rof", tag="rof", bufs=2)
                              nc.sync.dma_start(
                                  rof[:], att_out[h][fh][rt * 128 : (rt + 1) * 128, :]
                              )
                              dstv = ys[:, rt, cols]
                              nc.vector.tensor_tensor(dstv, dstv, rof[:], AL.add)
                          nc.vector.tensor_copy(
                              out=ysb[:, rt, fh, :], in_=ys[:, rt, cols]
                          )
                      # PE-transpose own y columns of this half -> yTo tiles
                      yTo = lp.tile([128, 8, RROWS], BF16, name="yTo", tag="yTo", bufs=2)
                      for rt in range(2):
                          for ec in range(8):
                              pth = psT.tile([128, 128], BF16, name="pth", tag="pt")
                              nc.tensor.transpose(
                                  pth[:],
                                  ysb[:, rt, fh, ec * 128 : (ec + 1) * 128],
                                  hidb[:],
                              )
                              if ec % 2 == 0:
                                  nc.vector.tensor_copy(
                                      out=yTo[:, ec, rt * 128 : (rt + 1) * 128],
                                      in_=pth[:],
                                  )
                              else:
                                  nc.scalar.copy(
                                      yTo[:, ec, rt * 128 : (rt + 1) * 128], pth[:]
                                  )
                      nc.sync.dma_start(
                          agt_in[fh].rearrange("(ec p) s -> p ec s", p=128),
                          yTo[:],
                      )
                      nc.gpsimd.collective_compute(
                          "AllGather",
                          AL.bypass,
                          replica_groups=RG,
                          ins=[agt_in[fh][:]],
                          outs=[agt_out[fh][:]],
                      )
                      # scatter gathered yT chunks into the full hT tile
                      for c in range(NCORES):
                          nc.sync.dma_start(
                              hT[:, fh * 8 : (fh + 1) * 8, c * RROWS : (c + 1) * RROWS],
                              agt_out[fh][
                                  c * (E // 2) : (c + 1) * (E // 2), :
                              ].rearrange("(eo p) s -> p eo s", p=128),
                          )

                  _stats_ln(nc, tc, lp, psT, ys, onesc, onesr, st1_in, st1_out, RG)
                  bc = _ln_scalars(nc, lp, psT, onesr, st1_out)

                  # rowwise h for own rows (LN2 residual input)
                  lngt = midp.tile([128, 2, E], BF16, name="lngt")
                  nc.sync.dma_start(lngt[:], lng_d.ap().rearrange("(t p) e -> p t e", p=128))
                  lnbt = midp.tile([128, 2, E], BF16, name="lnbt")
                  nc.sync.dma_start(lnbt[:], lnb_d.ap().rearrange("(t p) e -> p t e", p=128))
                  ht_f32 = lp.tile([128, E], F32, name="ht_f32", tag="htf", bufs=1)
                  for rt in range(2):
                      nc.scalar.activation(
                          ht_f32[:],
                          ys[:, rt, :],
                          AF.Identity,
                          bias=bc[:, 0:1],
                          scale=bc[:, 1:2],
                      )
                      nc.vector.tensor_tensor(
                          ht_f32[:], ht_f32[:], lngt[:, rt, :], AL.mult
                      )
                      nc.vector.tensor_tensor(
                          h_own[:, rt, :], ht_f32[:], lnbt[:, rt, :], AL.add
                      )

                  # full-y^T LN1 affine (chunked per eo; early chunks only
                  # depend on the first half's AllGather)
                  with tc.tile_pool(name="aff", bufs=2) as gb:
                      for eo in range(16):
                          gch = gb.tile([128, S], BF16, name="gch", tag="gch")
                          nc.sync.dma_start(gch[:], gT_d[:, eo, :])
                          bch = gb.tile([128, S], BF16, name="bch", tag="bch")
                          nc.sync.dma_start(bch[:], bT_d[:, eo, :])
                          tmpf = gb.tile([128, S], F32, name="tmpf", tag="tmpf")
                          nc.scalar.activation(
                              tmpf[:],
                              hT[:, eo, :],
                              AF.Identity,
                              bias=bc[:, 0:1],
                              scale=bc[:, 1:2],
                          )
                          nc.vector.tensor_tensor(tmpf[:], tmpf[:], gch[:], AL.mult)
                          nc.vector.tensor_tensor(hT[:, eo, :], tmpf[:], bch[:], AL.add)

              # =========== FFN (hidden shard 1024) ===========
              with tc.tile_pool(name="ffn", bufs=1) as fp, nc.named_scope("ffn"):
                  zT = fp.tile([128, 8, S], BF16, name="zT")
                  with tc.tile_pool(name="wst", bufs=2) as wst:
                      for ft in range(8):
                          w1b = wst.tile([128, 2048], BF16, name="w1b", tag="w1b")
                          nc.sync.dma_start(w1b[:], w1_d[:, ft, :])
                          pzs = [
                              ps512.tile([128, 512], F32, name=f"pz{sb}", tag="p512")
                              for sb in range(4)
                          ]
                          for eo in range(16):
                              for sb in range(4):
                                  nc.tensor.matmul(
                                      pzs[sb][:],
                                      w1b[:, eo * 128 : (eo + 1) * 128],
                                      hT[:, eo, sb * 512 : (sb + 1) * 512],
                                      start=(eo == 0),
                                      stop=(eo == 15),
                                  )
                          for sb in range(4):
                              nc.scalar.activation(
                                  zT[:, ft, sb * 512 : (sb + 1) * 512],
                                  pzs[sb][:],
                                  AF.Relu,
                                  bias=b1_sb[:, ft : ft + 1],
                              )
                      for eb in range(4):
                          w2b = wst.tile([128, 8, 512], BF16, name="w2b", tag="w2b")
                          nc.sync.dma_start(w2b[:], w2_d[:, :, eb, :])
                          for i in range(15, -1, -1):
                              pf = ps512.tile([128, 512], F32, name="pf", tag="p512")
                              for fc in range(8):
                                  nc.tensor.matmul(
                                      pf[:],
                                      zT[:, fc, i * 128 : (i + 1) * 128],
                                      w2b[:, fc, :],
                                      start=(fc == 0),
                                      stop=(fc == 7),
                                  )
                              fstg = wst.tile([128, 512], F16, name="fstg", tag="fstg", bufs=4)
                              nc.scalar.activation(fstg[:], pf[:], AF.Copy)
                              nc.sync.dma_start(
                                  ffn_in[eb][i * 128 : (i + 1) * 128, :], fstg[:]
                              )
                          nc.gpsimd.collective_compute(
                              "ReduceScatter",
                              AL.add,
                              replica_groups=RG,
                              ins=[ffn_in[eb][:]],
                              outs=[ffn_out[eb][:]],
                          )

              # =========== LN2 + output ===========
              with tc.tile_pool(name="ln2", bufs=1) as l2, nc.named_scope("ln2"):
                  ys2 = l2.tile([128, 2, E], F32, name="ys2")
                  yb2t = l2.tile([128, E], F32, name="yb2t")
                  nc.sync.dma_start(yb2t[:], yb2_d[:])
                  for rt in range(2):
                      nc.vector.tensor_tensor(
                          ys2[:, rt, :], h_own[:, rt, :], yb2t[:], AL.add
                      )
                      for eb in range(4):
                          fot = l2.tile([128, 512], F16, name="fot", tag="fot", bufs=2)
                          nc.sync.dma_start(
                              fot[:], ffn_out[eb][rt * 128 : (rt + 1) * 128, :]
                          )
                          dstv = ys2[:, rt, eb * 512 : (eb + 1) * 512]
                          nc.vector.tensor_tensor(dstv, dstv, fot[:], AL.add)

                  _stats_ln(nc, tc, l2, psT, ys2, onesc, onesr, st2_in, st2_out, RG)
                  bc2 = _ln_scalars(nc, l2, psT, onesr, st2_out)
                  lngt2 = lngt
                  lnbt2 = lnbt
                  for rt in range(2):
                      ot = l2.tile([128, E], F32, name="ot", tag="ot", bufs=2)
                      nc.scalar.activation(
                          ot[:],
                          ys2[:, rt, :],
                          AF.Identity,
                          bias=bc2[:, 0:1],
                          scale=bc2[:, 1:2],
                      )
                      nc.vector.tensor_tensor(ot[:], ot[:], lngt2[:, rt, :], AL.mult)
                      nc.vector.tensor_tensor(ot[:], ot[:], lnbt2[:, rt, :], AL.add)
                      nc.sync.dma_start(out_d[rt * 128 : (rt + 1) * 128, :], ot[:])

    nc.compile()
    return nc


def _stats_ln(nc, tc, pool, psT, ys, onesc, onesr, st_in, st_out, RG):
    """partial sum/sumsq of ys [128, 2, E] -> tiny fp32 AllReduce.

    Computed per (row-tile, column-half) so each partial only depends on the
    ReduceScatter chunks feeding that half (starts before the last RS lands).
    """
    parts = pool.tile([128, 8], F32, name="parts", tag="parts")
    sqs = pool.tile([128, E // 2], BF16, name="sqs", tag="sqs")
    for rt in range(2):
        for ch in range(2):
            idx = rt * 2 + ch
            ysl = ys[:, rt, ch * (E // 2) : (ch + 1) * (E // 2)]
            nc.vector.tensor_reduce(parts[:, idx : idx + 1], ysl, axis=AX.X, op=AL.add)
            nc.scalar.activation(
                sqs[:], ysl, AF.Square, accum_out=parts[:, 4 + idx : 5 + idx]
            )
    pstat = psT.tile([128, 128], F32, name="pstat", tag="pt")
    nc.tensor.matmul(pstat[:1, :8], onesc[:, 0:1], parts[:], start=True, stop=True)
    st4s = pool.tile([1, 8], F32, name="st4s", tag="st4s")
    nc.vector.tensor_copy(out=st4s[:], in_=pstat[:1, :8])
    st4 = pool.tile([1, 8], F32, name="st4", tag="st4")
    nc.vector.memset(st4[:], 0.0)
    nc.vector.tensor_reduce(st4[:, 0:1], st4s[:, 0:4], axis=AX.X, op=AL.add)
    nc.vector.tensor_reduce(st4[:, 1:2], st4s[:, 4:8], axis=AX.X, op=AL.add)
    nc.sync.dma_start(st_in[:], st4[:])
    nc.gpsimd.collective_compute(
        "AllReduce", AL.add, replica_groups=RG, ins=[st_in[:]], outs=[st_out[:]]
    )


def _ln_scalars(nc, pool, psT, onesr, st_out):
    """AllReduced (sum, sumsq) -> bc [128, 2] = (-m*rstd, rstd) broadcast."""
    so = pool.tile([1, 8], F32, name="so", tag="so")
    nc.sync.dma_start(so[:], st_out[:])
    sc = pool.tile([1, 8], F32, name="sc", tag="sc")
    # sc0 = m, sc1 = E[y^2], sc2 = m^2, sc3 = var, sc4 = rstd, sc5 = -m*rstd
    nc.scalar.mul(sc[:, 0:1], so[:, 0:1], 1.0 / NTOT)
    nc.scalar.mul(sc[:, 1:2], so[:, 1:2], 1.0 / NTOT)
    nc.scalar.activation(sc[:, 2:3], sc[:, 0:1], AF.Square)
    nc.vector.tensor_tensor(sc[:, 3:4], sc[:, 1:2], sc[:, 2:3], AL.subtract)
    nc.vector.tensor_scalar_add(sc[:, 2:3], sc[:, 3:4], EPS)  # var + eps
    # rstd = exp(-0.5 * ln(var + eps)) (keeps ACT on the exp/ln table)
    nc.scalar.activation(sc[:, 6:7], sc[:, 2:3], AF.Ln)
    nc.scalar.activation(sc[:, 4:5], sc[:, 6:7], AF.Exp, scale=-0.5)
    nc.vector.tensor_tensor(sc[:, 7:8], sc[:, 0:1], sc[:, 4:5], AL.mult)
    nc.scalar.mul(sc[:, 5:6], sc[:, 7:8], -1.0)
    s2 = pool.tile([1, 2], F32, name="s2", tag="s2")
    nc.vector.tensor_copy(out=s2[:, 0:1], in_=sc[:, 5:6])
    nc.vector.tensor_copy(out=s2[:, 1:2], in_=sc[:, 4:5])
    pb = psT.tile([128, 128], F32, name="pb", tag="pt")
    nc.tensor.matmul(pb[:, :2], onesr[:], s2[:], start=True, stop=True)
    bc = pool.tile([128, 2], F32, name="bc", tag="bc")
    nc.vector.tensor_copy(out=bc[:], in_=pb[:, :2])
    return bc


_NC_CACHE = None


def _get_nc():
    global _NC_CACHE
    if _NC_CACHE is None:
        _NC_CACHE = _build()
    return _NC_CACHE


_SHARED_CACHE = None


def _shared_prep(inputs):
    """Host tensors identical across cores (built once)."""
    global _SHARED_CACHE
    f32 = np.float32
    x = np.ascontiguousarray(inputs["input"], dtype=f32)
    ln_g, ln_b = inputs["ln_g"], inputs["ln_b"]
    bv, b2 = inputs["bv"], inputs["b2"]
    jj, tp, sf = np.meshgrid(
        np.arange(4), np.arange(128), np.arange(512), indexing="ij"
    )
    mask = ((128 * jj + tp) <= sf).astype(_bf16)
    xtb = np.ascontiguousarray(
        x.T.reshape(16, 128, S).transpose(1, 0, 2).astype(_bf16)
    )
    gT = np.ascontiguousarray(
        np.asarray(ln_g, f32).T.reshape(16, 128, S).transpose(1, 0, 2).astype(_bf16)
    )
    bT = np.ascontiguousarray(
        np.asarray(ln_b, f32).T.reshape(16, 128, S).transpose(1, 0, 2).astype(_bf16)
    )
    return {
        "x": x,
        "xtb": xtb,
        "gT": gT,
        "bT": bT,
        "mask": np.ascontiguousarray(mask.transpose(1, 0, 2)),
        "ident": np.eye(128, dtype=f32).astype(_bf16),
        "ones": np.ones((128, 8), dtype=f32),
        "onesr": np.ones((1, 128), dtype=f32),
        "yb1": np.ascontiguousarray(
            np.broadcast_to(np.asarray(bv, f32).sum(axis=0), (128, E)), dtype=f32
        ),
        "yb2": np.ascontiguousarray(
            np.broadcast_to(np.asarray(b2, f32), (128, E)), dtype=f32
        ),
    }


def _prep_core(c, inputs, shared=None):
    f32 = np.float32
    if shared is None:
        shared = _shared_prep(inputs)
    x = shared["x"]
    Wq, Wk, Wv = inputs["Wq"], inputs["Wk"], inputs["Wv"]
    bq, bk = inputs["bq"], inputs["bk"]
    W1, b1, W2 = inputs["W1"], inputs["b1"], inputs["W2"]
    ln_g, ln_b = inputs["ln_g"], inputs["ln_b"]
    h0 = c * HPC
    wqt = np.ascontiguousarray(
        np.stack(
            [Wq[h0 + h].reshape(16, 128, KD).transpose(1, 0, 2) for h in range(HPC)]
        ).astype(_bf16)
    )
    wkt = np.ascontiguousarray(
        np.stack(
            [Wk[h0 + h].reshape(16, 128, KD).transpose(1, 0, 2) for h in range(HPC)]
        ).astype(_bf16)
    )
    wvt = np.ascontiguousarray(
        np.stack(
            [Wv[h0 + h].reshape(16, 128, E).transpose(1, 0, 2) for h in range(HPC)]
        ).astype(_bf16)
    )
    W1s = np.asarray(W1)[:, c * FSH : (c + 1) * FSH]
    w1t = np.ascontiguousarray(
        W1s.reshape(16, 128, 8, 128).transpose(1, 2, 0, 3).reshape(128, 8, 2048).astype(_bf16)
    )
    W2s = np.asarray(W2)[c * FSH : (c + 1) * FSH, :]
    w2t = np.ascontiguousarray(
        W2s.reshape(8, 128, 4, 512).transpose(1, 0, 2, 3).astype(_bf16)
    )
    bqs = np.ascontiguousarray((np.asarray(bq, f32)[h0 : h0 + HPC] * ISCALE).T, dtype=f32)
    bks = np.ascontiguousarray(np.asarray(bk, f32)[h0 : h0 + HPC].T, dtype=f32)
    b1s = np.ascontiguousarray(
        np.asarray(b1, f32)[c * FSH : (c + 1) * FSH].reshape(8, 128).T, dtype=f32
    )
    rows = slice(c * RROWS, (c + 1) * RROWS)
    return {
        "xtb": shared["xtb"],
        "gT": shared["gT"],
        "bT": shared["bT"],
        "mask": shared["mask"],
        "ident": shared["ident"],
        "ones": shared["ones"],
        "onesr": shared["onesr"],
        "yb1": shared["yb1"],
        "yb2": shared["yb2"],
        "wqt": wqt,
        "wkt": wkt,
        "wvt": wvt,
        "w1t": w1t,
        "w2t": w2t,
        "bqs": bqs,
        "bks": bks,
        "b1s": b1s,
        "xr": np.ascontiguousarray(x[rows], dtype=f32),
        "lngr": np.ascontiguousarray(np.asarray(ln_g, f32)[rows].astype(_bf16)),
        "lnbr": np.ascontiguousarray(np.asarray(ln_b, f32)[rows].astype(_bf16)),
    }


def kernel(**inputs):
    nc = _get_nc()
    inputs = {k: np.asarray(v, dtype=np.float32) for k, v in inputs.items()}
    shared = _shared_prep(inputs)
    in_maps = [_prep_core(c, inputs, shared) for c in range(NCORES)]
    res = run_bass_kernel_spmd(nc, in_maps, core_ids=list(range(NCORES)))
    out = np.concatenate([res.results[c]["out"] for c in range(NCORES)], axis=0)
    return np.ascontiguousarray(out, dtype=np.float32)
